# revision 12
# baseline (speedup 1.0000x reference)
"""Hausdorff distance kernel for Trainium2 (8 NeuronCores).

Reference computes, per sample n (N=2), on a 20^3 voxel grid (V=8000):
  d[i,j]   = Euclidean distance between voxel centers (coords / 20)
  min_to_B = min over j in B of d[i,j]
  distA    = max over i in Aonly of min_to_B   (Aonly = A & ~B)
  (symmetrically distB), haus_n = max(distA, distB); output = mean_n haus_n.

Strategy (separable-grid retrieval):
 - min over j in B of d2(i,j) = min over (x,y) lines of
     dxy2(i, line) + gB[line, z_i],
   where gB[line, iz] = min over jz in B(line) of (iz-jz)^2 is a 1D EDT
   along z that the host precomputes per line (cheap numpy re-encoding of
   the B mask).  The device retrieval is then rows x 400 line-candidates
   instead of rows x 4000 point-candidates: ~10x less PE/DVE work than
   the full pairwise matrix.
 - Per 128-row tile, ONE K=24 matmul: lhsT rows [sq_xy_i, 1, -2x_i,
   -2y_i, onehot(z_i) x 20], rhs rows [1, sq_xy_j, x_j, y_j, gB^T].
   PSUM[i, line] = dxy2 + gB[line, z_i]; DVE row-min -> min d2 (exact:
   all inputs are small integers, fp16-exact; f32 PSUM sums exact).
 - 4 (sample, direction) problems; core c handles (problem c//2,
   row-half c%2).  Final sqrt + max on host.
"""

import sys
import functools

import numpy as np

for _p in ("/opt/trn_rl_repo",):
    if _p not in sys.path:
        sys.path.insert(0, _p)

from concourse import bass, mybir, tile  # noqa: E402
from concourse.bass_utils import run_bass_kernel_spmd  # noqa: E402

D = H = W = 20
V = D * H * W
NLINE = D * H  # 400 (x,y) lines
KDIM = 4 + W   # 24 contraction rows
N_CORES = 8
BIG = 1e9
BIGLN = 16384.0  # empty-line sentinel; power of two, fp16-exact
F16 = mybir.dt.float16
F32 = mybir.dt.float32


def _coords_int():
    x, y, z = np.meshgrid(np.arange(D), np.arange(H), np.arange(W), indexing="ij")
    return np.stack([x, y, z], axis=-1).reshape(V, 3).astype(np.float64)


_COORDS = _coords_int()
# static quadratic kernel (iz - jz)^2
_Q = (np.arange(W)[:, None] - np.arange(W)[None, :]).astype(np.float64) ** 2


def _round_up(x, m):
    return max(m, ((int(x) + m - 1) // m) * m)


@functools.lru_cache(maxsize=None)
def _build(r_cap):
    """Raw-bass program: per-core [24,r_cap] x [24,400] -> row mins [128, r_cap//128].

    Hand-rolled semaphores (Tile's end-of-kernel barrier costs ~10us).
    Input DMA row-split across 3 engine HWDGE queues (descriptor-gen time
    scales with partition count; 3 engines issue in parallel).  Pipeline:
    PE fills PSUM banks round-robin; Pool pair-mins each bank [128,400] ->
    [128,200] f16 (values <= 1083 finite / >= 16384 sentinel stay exact);
    DVE min-reduces the f16 strip at 2 elem/cycle.  Keeps every engine
    under the PE's 333ns/tile cadence.
    """
    nrt = r_cap // 128
    nc = bass.Bass()
    # single input tensor: [24, r_cap] lhsT block then [24, 400] rhs block
    inp_d = nc.declare_dram_parameter("inp", [KDIM, r_cap + NLINE], F16, isOutput=False)
    out_d = nc.declare_dram_parameter("out", [128, nrt], F32, isOutput=True)

    with (
        nc.sbuf_tensor("inp_t", [KDIM, r_cap + NLINE], F16) as inp_t,
        nc.sbuf_tensor("allmin", [128, nrt], F32) as allmin,
        nc.psum_tensor("ps", [128, 8, 512], F32) as ps,
        nc.semaphore("in_sem") as in_sem,
        nc.semaphore("pe_sem") as pe_sem,
        nc.semaphore("dve_sem") as dve_sem,
        nc.semaphore("out_sem") as out_sem,
    ):
        lhsT_t = inp_t.ap()[:, :r_cap]
        rhs_t = inp_t.ap()[:, r_cap:]

        nc.sync.dma_start(
            out=inp_t.ap(), in_=inp_d[:], single_packet=True
        ).then_inc(in_sem, 16)

        nc.tensor.wait_ge(in_sem, 16)
        for k in range(nrt):
            if k >= 8:
                nc.tensor.wait_ge(dve_sem, k - 8 + 1)
            nc.tensor.matmul(
                ps.ap()[:, k % 8, :NLINE],
                lhsT_t[:, k * 128 : (k + 1) * 128],
                rhs_t,
            ).then_inc(pe_sem, 1)

        for k in range(nrt):
            nc.vector.wait_ge(pe_sem, k + 1)
            nc.vector.tensor_reduce(
                allmin.ap()[:, k : k + 1],
                ps.ap()[:, k % 8, :NLINE],
                axis=mybir.AxisListType.X,
                op=mybir.AluOpType.min,
            ).then_inc(dve_sem, 1)

        # No wait on out_sem: the walrus epilogue's per-engine DRAINs already
        # guarantee DMA-queue quiescence before NEFF exit, so the out-DMA
        # trigger/transfer latency overlaps the (fixed ~7us) teardown.
        nc.sync.wait_ge(dve_sem, nrt)
        nc.sync.dma_start(
            out=out_d[:], in_=allmin.ap(), single_packet=True
        ).then_inc(out_sem, 16)
    return nc


def _make_lhsT(pts, r_cap):
    """pts: [k,3] integer coords. Rows: [sq_xy, 1, -2x, -2y, onehot(z)x20]."""
    k = len(pts)
    if k == 0:
        pts = np.zeros((1, 3))
        k = 1
    pad = np.concatenate([pts, np.broadcast_to(pts[0], (r_cap - k, 3))], axis=0)
    arr = np.zeros((KDIM, r_cap), np.float16)
    arr[0] = pad[:, 0] ** 2 + pad[:, 1] ** 2
    arr[1] = 1.0
    arr[2] = -2.0 * pad[:, 0]
    arr[3] = -2.0 * pad[:, 1]
    arr[4 + pad[:, 2].astype(np.int64), np.arange(r_cap)] = 1.0
    return arr


def _make_rhs(mask):
    """mask: [V] bool of the candidate set B.
    Rows: [1, sq_xy_j, x_j, y_j, gB^T] with gB the per-line 1D z-EDT."""
    maskval = np.where(mask.reshape(D, H, W), 0.0, BIGLN)
    # gB[x, y, iz] = min_jz maskval[x, y, jz] + (iz-jz)^2
    g = (maskval[:, :, None, :] + _Q[None, None, :, :]).min(axis=3)
    arr = np.empty((KDIM, NLINE), np.float16)
    xj = _COORDS[::W, 0]
    yj = _COORDS[::W, 1]
    arr[0] = 1.0
    arr[1] = xj**2 + yj**2
    arr[2] = xj
    arr[3] = yj
    arr[4:] = g.reshape(NLINE, W).T
    return arr


def kernel(predict, target):
    predict = np.asarray(predict)
    target = np.asarray(target)
    n = predict.shape[0]
    im_a = np.round(predict.reshape(n, V)) != 0
    im_b = np.round(target.reshape(n, V)) != 0

    # 2*n directed problems: (rows = one-sided points, cols = other full set)
    probs = []
    for s in range(n):
        ma, mb = im_a[s], im_b[s]
        probs.append((_COORDS[ma & ~mb], mb))  # distA direction
        probs.append((_COORDS[mb & ~ma], ma))  # distB direction
    n_probs = len(probs)
    halves = N_CORES // n_probs  # 2 for N=2

    # per-core row slices (contiguous split into `halves` chunks)
    core_rows = []
    for p in range(n_probs):
        rows = probs[p][0]
        k = len(rows)
        per = -(-max(k, 1) // halves)
        for h in range(halves):
            core_rows.append(rows[h * per : (h + 1) * per])

    r_cap = _round_up(max((len(r) for r in core_rows), default=1), 128)

    nc = _build(r_cap)
    in_maps = []
    rhs_cache = {}
    for c in range(N_CORES):
        p = c // halves
        if p not in rhs_cache:
            rhs_cache[p] = _make_rhs(probs[p][1])
        inp = np.concatenate([_make_lhsT(core_rows[c], r_cap), rhs_cache[p]], axis=1)
        in_maps.append({"inp": inp})
    results = run_bass_kernel_spmd(nc, in_maps, list(range(N_CORES))).results

    # out[i, rt] = min d2 for row rt*128+i  -> flatten to [r_cap]
    core_mins = [np.asarray(results[c]["out"]).T.reshape(-1) for c in range(N_CORES)]

    dists = np.empty(n_probs, np.float64)
    for p in range(n_probs):
        rows, cmask = probs[p]
        if len(rows) == 0:
            dists[p] = 0.0
            continue
        if not cmask.any():
            # reference: min_to_X == BIG everywhere -> directed dist = BIG;
            # (distB's 999.0 special case is applied below)
            dists[p] = BIG
            continue
        parts = []
        for h in range(halves):
            cr = core_rows[p * halves + h]
            if len(cr):
                parts.append(core_mins[p * halves + h][: len(cr)])
        d2max = max(float(x.max()) for x in parts)
        dists[p] = np.sqrt(d2max / 400.0)

    haus = np.empty(n, np.float64)
    for s in range(n):
        dist_a, dist_b = dists[2 * s], dists[2 * s + 1]
        ma, mb = im_a[s], im_b[s]
        if (mb & ~ma).any() and not ma.any():
            dist_b = 999.0
        haus[s] = max(dist_a, dist_b)
    return np.float32(haus.mean())
